# revision 3
# baseline (speedup 1.0000x reference)
"""v3: v2 + bf16 x-echo in phase A + gather-based scatter phase E.

E': tokens bucketed by dst block (F>>7). Rank-within-bucket via DVE prefix
scan; slot = 640*bucket + rank - 1; inverse table via dma_scatter_add of
256B payload rows (token_id+1) into a zeroed [5120, 64] DRAM table; read
back, clamp dummies (x-gather -> row 0 masked by zero S; S-gather -> zeros
row 4096). Per bucket: dma_gather 640 x rows (bf16 echo) + 640 S-slices
(one-hot rows restricted to the bucket's 128 dst cols); 5 stationary
matmuls per 512-chunk accumulate PSUM; scale by 1/count; DMA out.
PE cost ~205k cycles vs ~1M for the dense one-hot scatter.
"""

import numpy as np

import concourse.bacc as bacc
import concourse.bass as bass
import concourse.mybir as mybir
import concourse.tile as tile
from concourse.bass_utils import run_bass_kernel_spmd
from concourse.masks import make_identity

F32 = mybir.dt.float32
BF16 = mybir.dt.bfloat16
F16 = mybir.dt.float16
U32 = mybir.dt.uint32
I16 = mybir.dt.int16

P = 128
NT = 16
C = 4096
D = 128
PTOK = 4096
N1 = 2048
N2 = 1024
NSLOT = 5120      # 8 buckets x 640 slots
AL = mybir.AluOpType

_LVL = {"A": 0, "B": 1, "C": 2, "D": 3, "T": 4, "E": 5}


def _newton_rsqrt(nc, pool, y, n2, shape, pfx):
    t = pool.tile(shape, F32, tag=f"{pfx}newt", name=f"{pfx}newt")
    for _ in range(2):
        nc.vector.tensor_mul(t[:], y, y)
        nc.vector.tensor_mul(t[:], t[:], n2)
        nc.vector.tensor_scalar(t[:], t[:], -0.5, 1.5, AL.mult, AL.add)
        nc.vector.tensor_mul(y, y, t[:])


def build_kernel(stop_after="E", reps=1):
    lvl = _LVL[stop_after]
    nc = bacc.Bacc(None, target_bir_lowering=False)
    x = nc.dram_tensor("x", [PTOK, C], F32, kind="ExternalInput")
    out = nc.dram_tensor("out", [N2, C], F32, kind="ExternalOutput")
    g_dram = nc.dram_tensor("g_scratch", [N1], F32, kind="Internal")
    cnt_dram = nc.dram_tensor("cnt_scratch", [N2], F32, kind="Internal")
    i1_dram = nc.dram_tensor("i1_scratch", [N1], F32, kind="Internal")
    blk_dram = nc.dram_tensor("blk_scratch", [PTOK], F32, kind="Internal")
    slot_dram = nc.dram_tensor("slot_scratch", [PTOK], F32, kind="Internal")
    gidx_dram = nc.dram_tensor("gidx_scratch", [NSLOT, 64], F32,
                               kind="Internal")
    echo = nc.dram_tensor("echo", [PTOK, C], BF16, kind="Internal")
    s_dram = nc.dram_tensor("s_onehot", [PTOK + 1, N2], BF16, kind="Internal")

    xv = x[:].rearrange("(t p o) c -> o t p c", t=NT, p=P, o=2)
    ev = echo[:].rearrange("(t p o) c -> o t p c", t=NT, p=P, o=2)
    sv2 = s_dram[0:PTOK].rearrange("(t p o) c -> o t p c", t=NT, p=P, o=2)
    sv4 = s_dram[0:PTOK].rearrange("(q p r) c -> r q p c", q=8, p=P, r=4)

    with tile.TileContext(nc) as tc:
        with tc.tile_pool(name="const", bufs=1) as cpool:
            cst = {}
            ident = cpool.tile([P, P], F32)
            make_identity(nc, ident[:])
            cst["ident"] = ident
            ident_bf = cpool.tile([P, P], BF16)
            nc.vector.tensor_copy(ident_bf[:], ident[:])
            cst["ident_bf"] = ident_bf
            iota2048 = cpool.tile([P, N1], F32)
            nc.gpsimd.iota(iota2048[:], pattern=[[1, N1]], base=0,
                           channel_multiplier=0,
                           allow_small_or_imprecise_dtypes=True)
            cst["iota2048"] = iota2048
            iota1024 = cpool.tile([P, N2], F32)
            nc.gpsimd.iota(iota1024[:], pattern=[[1, N2]], base=0,
                           channel_multiplier=0,
                           allow_small_or_imprecise_dtypes=True)
            cst["iota1024"] = iota1024
            ones_col = cpool.tile([P, 1], F32)
            nc.vector.memset(ones_col[:], 1.0)
            cst["ones_col"] = ones_col
            ones_col_bf = cpool.tile([P, 1], BF16)
            nc.vector.memset(ones_col_bf[:], 1.0)
            cst["ones_col_bf"] = ones_col_bf
            ones_row1 = cpool.tile([1, P], F32)
            nc.vector.memset(ones_row1[:], 1.0)
            cst["ones_row1"] = ones_row1
            iota_pcol = cpool.tile([P, NT], F32)
            nc.gpsimd.iota(iota_pcol[:], pattern=[[P, NT]], base=0,
                           channel_multiplier=1,
                           allow_small_or_imprecise_dtypes=True)
            cst["iota_pcol"] = iota_pcol
            iota8 = cpool.tile([8, 1], F32)
            nc.gpsimd.iota(iota8[:], pattern=[[0, 1]], base=0,
                           channel_multiplier=1,
                           allow_small_or_imprecise_dtypes=True)
            cst["iota8"] = iota8
            for rep in range(reps):
                _body(nc, tc, lvl, f"r{rep}_" if reps > 1 else "",
                      out, g_dram, cnt_dram, i1_dram, blk_dram, slot_dram,
                      gidx_dram, echo, s_dram, xv, ev, sv2, sv4, cst)
    nc.finalize()
    return nc


def _body(nc, tc, lvl, fx, out, g_dram, cnt_dram, i1_dram, blk_dram,
          slot_dram, gidx_dram, echo, s_dram, xv, ev, sv2, sv4, cst):
    with tc.tile_pool(name=f"{fx}small", bufs=1) as spool:
        idx1f = spool.tile([P, NT], F32)
        F_all = spool.tile([P, 2 * NT], F32)
        s2r = spool.tile([P, 8], F32)
        idx2f = spool.tile([P, 8], F32)
        done = _phases_abcd(nc, tc, lvl, fx, out, g_dram, i1_dram, xv, ev,
                            cst, spool, idx1f, F_all, idx2f)
        if not done:
            return
        _phase_e(nc, tc, lvl, fx, out, cnt_dram, blk_dram, slot_dram,
                 gidx_dram, echo, s_dram, sv2, sv4, cst, F_all, idx2f, s2r)


def _phases_abcd(nc, tc, lvl, fx, out, g_dram, i1_dram, xv, ev, cst,
                 spool, idx1f, F_all, idx2f):
    ident = cst["ident"]
    iota2048 = cst["iota2048"]
    ones_col = cst["ones_col"]
    ones_row1 = cst["ones_row1"]
    iota_pcol = cst["iota_pcol"]

    def head_sum(pool, xt, nm):
        """Out-of-place first level (keeps xt intact for the echo DMA),
        then in-place tree on the half-size scratch."""
        ht = pool.tile([P, N1], F32, tag="ht", name=nm)
        nc.vector.tensor_add(ht[:], xt[:, :N1], xt[:, N1:])
        w = N1
        while w > D:
            h = w // 2
            nc.vector.tensor_add(ht[:, :h], ht[:, :h], ht[:, h:w])
            w = h
        return ht

    with tc.tile_pool(name=f"{fx}big", bufs=1) as bpool:
        mx_even = bpool.tile([P, N1], F32)
        mxT_even = bpool.tile([P, N1], F32)
        mxT_odd = bpool.tile([P, N1], F32)
        mxT_odd_n = bpool.tile([P, N1], F32)
        macc_eT = bpool.tile([P, N2], F32)
        macc_oT = bpool.tile([P, N2], F32)
        macc_oT_n = bpool.tile([P, N2], F32)
        idx_b1 = spool.tile([P, NT * 8], U32)
        idx_b2 = spool.tile([P, 8 * 8], U32)

        # ========== A-odd ==========
        with (
            tc.tile_pool(name=f"{fx}xa", bufs=4) as xa,
            tc.tile_pool(name=f"{fx}ho", bufs=4) as ho,
            tc.tile_pool(name=f"{fx}psA", bufs=2, space="PSUM") as psA,
        ):
            for ti in range(NT):
                xt = xa.tile([P, C], F32, tag="xt", name=f"{fx}xo{ti}")
                nc.sync.dma_start(xt[:], xv[1, ti])
                nc.gpsimd.dma_start(ev[1, ti], xt[:])
                ht = head_sum(ho, xt, f"{fx}ho{ti}")
                pt = psA.tile([P, P], F32, tag="tr", name=f"{fx}tro{ti}")
                nc.tensor.transpose(pt[:], ht[:, :D], ident[:])
                nc.scalar.copy(mxT_odd[:, ti * D:(ti + 1) * D], pt[:])

            with (
                tc.tile_pool(name=f"{fx}nb", bufs=1) as nb,
                tc.tile_pool(name=f"{fx}psN", bufs=2, space="PSUM") as psN,
            ):
                sq = nb.tile([P, N1], F32)
                nc.scalar.activation(sq[:], mxT_odd[:],
                                     mybir.ActivationFunctionType.Square)
                n2row = nb.tile([1, N1], F32)
                for jc in range(4):
                    pn = psN.tile([1, 512], F32, tag="n2",
                                  name=f"{fx}n2_{jc}")
                    nc.tensor.matmul(pn[:], ones_col[:],
                                     sq[:, jc * 512:(jc + 1) * 512],
                                     start=True, stop=True)
                    nc.scalar.copy(n2row[:, jc * 512:(jc + 1) * 512], pn[:])
                rinv = nb.tile([1, N1], F32)
                sqr = nb.tile([1, N1], F32)
                nc.scalar.activation(sqr[:], n2row[:],
                                     mybir.ActivationFunctionType.Sqrt)
                nc.vector.reciprocal(rinv[:], sqr[:])
                _newton_rsqrt(nc, nb, rinv[:], n2row[:], [1, N1], fx)
                for jc in range(4):
                    pb = psN.tile([P, 512], F32, tag="bc",
                                  name=f"{fx}bc_{jc}")
                    nc.tensor.matmul(pb[:], ones_row1[:],
                                     rinv[:, jc * 512:(jc + 1) * 512],
                                     start=True, stop=True)
                    nc.vector.tensor_mul(
                        mxT_odd_n[:, jc * 512:(jc + 1) * 512],
                        mxT_odd[:, jc * 512:(jc + 1) * 512], pb[:])

            # ===== A-even + scores1 + pmacc =====
            with (
                tc.tile_pool(name=f"{fx}sc1", bufs=2) as sc1,
                tc.tile_pool(name=f"{fx}ps1", bufs=2, space="PSUM") as ps1,
                tc.tile_pool(name=f"{fx}s1p", bufs=2) as s1p,
                tc.tile_pool(name=f"{fx}psM", bufs=1, space="PSUM") as psM,
            ):
                m8 = spool.tile([P, 8], F32)
                pmacc = psM.tile([P, N1], F32, tag="macc")
                for ti in range(NT):
                    xt = xa.tile([P, C], F32, tag="xt", name=f"{fx}xe{ti}")
                    nc.sync.dma_start(xt[:], xv[0, ti])
                    nc.gpsimd.dma_start(ev[0, ti], xt[:])
                    ht = head_sum(ho, xt, f"{fx}he{ti}")
                    nc.scalar.copy(mx_even[:, ti * D:(ti + 1) * D],
                                   ht[:, :D])
                    pt = psA.tile([P, P], F32, tag="tr", name=f"{fx}tre{ti}")
                    nc.tensor.transpose(pt[:], ht[:, :D], ident[:])
                    nc.scalar.copy(mxT_even[:, ti * D:(ti + 1) * D], pt[:])
                    ssb = sc1.tile([P, N1], F32, tag="ssb",
                                   name=f"{fx}sb{ti}")
                    for jc in range(4):
                        psc = ps1.tile([P, 512], F32, tag="sc",
                                       name=f"{fx}sc{ti}_{jc}")
                        nc.tensor.matmul(
                            psc[:], mxT_even[:, ti * D:(ti + 1) * D],
                            mxT_odd_n[:, jc * 512:(jc + 1) * 512],
                            start=True, stop=True)
                        nc.scalar.copy(ssb[:, jc * 512:(jc + 1) * 512],
                                       psc[:])
                    nc.vector.max(m8[:], ssb[:])
                    nc.vector.max_index(idx_b1[:, ti * 8:(ti + 1) * 8],
                                        m8[:], ssb[:])
                    nc.vector.tensor_copy(idx1f[:, ti:ti + 1],
                                          idx_b1[:, ti * 8:ti * 8 + 1])
                    s1t = s1p.tile([P, N1], F32, tag="s1",
                                   name=f"{fx}s1_{ti}")
                    nc.vector.tensor_single_scalar(
                        s1t[:], iota2048[:], idx1f[:, ti:ti + 1],
                        AL.is_equal)
                    for jc in range(4):
                        nc.tensor.matmul(
                            pmacc[:, jc * 512:(jc + 1) * 512],
                            mx_even[:, ti * D:(ti + 1) * D],
                            s1t[:, jc * 512:(jc + 1) * 512],
                            start=(ti == 0), stop=(ti == NT - 1),
                            skip_group_check=True)
                if lvl == 0:
                    nc.sync.dma_start(out[0:P, 0:N1], mx_even[:])
                    nc.sync.dma_start(out[0:P, N1:2 * N1], mxT_even[:])
                    nc.sync.dma_start(out[P:2 * P, 0:N1], mxT_odd[:])
                if lvl == 1:
                    nc.sync.dma_start(out[0:P, 0:NT], idx1f[:])
                    nc.sync.dma_start(out[0:P, 32:32 + N1], mxT_odd_n[:])
                if lvl < 2:
                    return False
                nc.vector.tensor_add(macc_eT[:], pmacc[:, ::2],
                                     mxT_odd[:, ::2])
                nc.vector.tensor_add(macc_oT[:], pmacc[:, 1::2],
                                     mxT_odd[:, 1::2])

            with (
                tc.tile_pool(name=f"{fx}nb2", bufs=1) as nb2,
                tc.tile_pool(name=f"{fx}psN2", bufs=2, space="PSUM") as psN2,
            ):
                sq2 = nb2.tile([P, N2], F32)
                nc.scalar.activation(sq2[:], macc_oT[:],
                                     mybir.ActivationFunctionType.Square)
                n2row2 = nb2.tile([1, N2], F32)
                for jc in range(2):
                    pn = psN2.tile([1, 512], F32, tag="n2",
                                   name=f"{fx}n2b_{jc}")
                    nc.tensor.matmul(pn[:], ones_col[:],
                                     sq2[:, jc * 512:(jc + 1) * 512],
                                     start=True, stop=True)
                    nc.scalar.copy(n2row2[:, jc * 512:(jc + 1) * 512],
                                   pn[:])
                rinv2 = nb2.tile([1, N2], F32)
                sqr2 = nb2.tile([1, N2], F32)
                nc.scalar.activation(sqr2[:], n2row2[:],
                                     mybir.ActivationFunctionType.Sqrt)
                nc.vector.reciprocal(rinv2[:], sqr2[:])
                _newton_rsqrt(nc, nb2, rinv2[:], n2row2[:], [1, N2],
                              fx + "b")
                for jc in range(2):
                    pb = psN2.tile([P, 512], F32, tag="bc",
                                   name=f"{fx}bcb_{jc}")
                    nc.tensor.matmul(pb[:], ones_row1[:],
                                     rinv2[:, jc * 512:(jc + 1) * 512],
                                     start=True, stop=True)
                    nc.vector.tensor_mul(
                        macc_oT_n[:, jc * 512:(jc + 1) * 512],
                        macc_oT[:, jc * 512:(jc + 1) * 512], pb[:])

            with (
                tc.tile_pool(name=f"{fx}sc2", bufs=2) as sc2,
                tc.tile_pool(name=f"{fx}ps2", bufs=2, space="PSUM") as ps2,
            ):
                m8b = spool.tile([P, 8], F32)
                for t2 in range(8):
                    ssb2 = sc2.tile([P, N2], F32, tag="ssb2",
                                    name=f"{fx}sb2_{t2}")
                    for jc in range(2):
                        psc = ps2.tile([P, 512], F32, tag="sc2",
                                       name=f"{fx}sc2_{t2}_{jc}")
                        nc.tensor.matmul(
                            psc[:], macc_eT[:, t2 * D:(t2 + 1) * D],
                            macc_oT_n[:, jc * 512:(jc + 1) * 512],
                            start=True, stop=True)
                        nc.scalar.copy(ssb2[:, jc * 512:(jc + 1) * 512],
                                       psc[:])
                    nc.vector.max(m8b[:], ssb2[:])
                    nc.vector.max_index(idx_b2[:, t2 * 8:(t2 + 1) * 8],
                                        m8b[:], ssb2[:])
            nc.vector.tensor_copy(idx2f[:], idx_b2[:, ::8])
            if lvl == 2:
                nc.sync.dma_start(out[0:P, 0:8], idx2f[:])
                nc.sync.dma_start(out[0:P, 8:8 + N2], macc_eT[:])
                nc.sync.dma_start(out[P:2 * P, 0:N2], macc_oT[:])
            if lvl < 3:
                return False

            # ================= D: compose F =================
            with (
                tc.tile_pool(name=f"{fx}cmp", bufs=1) as cmp,
                tc.tile_pool(name=f"{fx}s1d", bufs=2) as s1d,
                tc.tile_pool(name=f"{fx}psD", bufs=1, space="PSUM") as psD,
            ):
                g_row = cmp.tile([1, N1], F32)
                nc.gpsimd.iota(g_row[0:1, 1::2], pattern=[[1, N2]], base=0,
                               channel_multiplier=0,
                               allow_small_or_imprecise_dtypes=True)
                gv = g_dram[:].rearrange("(t p o) -> o p t", t=8, p=P, o=2)
                nc.sync.dma_start(gv[0], idx2f[:])
                gk = g_dram[:].rearrange("(k o) -> o k", o=2)
                nc.sync.dma_start(g_row[0:1, 0::2], gk[0][None, :])
                nc.sync.dma_start(g_dram[:][None, :], g_row[:])
                gf = g_dram[:].rearrange("(t p) -> p t", t=NT, p=P)
                nc.sync.dma_start(F_all[:, NT:2 * NT], gf)
                i1d = i1_dram[:].rearrange("(t p) -> p t", t=NT, p=P)
                nc.sync.dma_start(i1d, idx1f[:])
                i1row = cmp.tile([1, N1], F32)
                nc.sync.dma_start(i1row[:], i1_dram[:][None, :])
                idx1_bc = cmp.tile([P, N1], F32)
                for jc in range(4):
                    pb = psD.tile([P, 512], F32, tag="gb",
                                  name=f"{fx}gb{jc}")
                    nc.tensor.matmul(pb[:], ones_row1[:],
                                     i1row[:, jc * 512:(jc + 1) * 512],
                                     start=True, stop=True)
                    nc.scalar.copy(idx1_bc[:, jc * 512:(jc + 1) * 512],
                                   pb[:])
                g16 = cmp.tile([P, NT], F16)
                nc.vector.tensor_copy(g16[:], F_all[:, NT:2 * NT])
                pfr = [psD.tile([1, 512], F32, tag=f"pfr{c}",
                                name=f"{fx}pfr{c}") for c in range(4)]
                for jt in range(NT):
                    s1tt = s1d.tile([P, N1], F16, tag="s1d",
                                    name=f"{fx}s1tt_{jt}")
                    nc.vector.tensor_single_scalar(
                        s1tt[:], idx1_bc[:], iota_pcol[:, jt:jt + 1],
                        AL.is_equal)
                    for ic in range(4):
                        nc.tensor.matmul(
                            pfr[ic][:], g16[:, jt:jt + 1],
                            s1tt[:, ic * 512:(ic + 1) * 512],
                            start=(jt == 0), stop=(jt == NT - 1),
                            skip_group_check=True)
                fe_row = cmp.tile([1, N1], F32)
                for ic in range(4):
                    nc.scalar.copy(fe_row[:, ic * 512:(ic + 1) * 512],
                                   pfr[ic][:])
                nc.sync.dma_start(i1_dram[:][None, :], fe_row[:])
                nc.sync.dma_start(
                    F_all[:, 0:NT],
                    i1_dram[:].rearrange("(t p) -> p t", t=NT, p=P))
            if lvl == 3:
                nc.sync.dma_start(out[0:P, 0:2 * NT], F_all[:])
            return lvl >= 4


def _phase_e(nc, tc, lvl, fx, out, cnt_dram, blk_dram, slot_dram, gidx_dram,
             echo, s_dram, sv2, sv4, cst, F_all, idx2f, s2r):
    ident_bf = cst["ident_bf"]
    iota1024 = cst["iota1024"]
    ones_col = cst["ones_col"]
    ones_col_bf = cst["ones_col_bf"]
    ones_row1 = cst["ones_row1"]
    iota8 = cst["iota8"]

    with tc.tile_pool(name=f"{fx}ix", bufs=1) as ixp:
        # idx tables live in their own pool: 128 partitions (HW wrap layout
        # uses the first 16), int16.
        idxT_x = ixp.tile([P, 320], I16)
        idxT_s = ixp.tile([P, 320], I16)

        with (
            tc.tile_pool(name=f"{fx}tb", bufs=1) as tb,
            tc.tile_pool(name=f"{fx}psT", bufs=2, space="PSUM") as psT,
        ):
            # ---- S one-hot tiles -> s_dram; counts -> s2r ----
            with (
                tc.tile_pool(name=f"{fx}sf", bufs=1) as sfp,
                tc.tile_pool(name=f"{fx}sid", bufs=2) as sidp,
            ):
                sf_tiles = []
                for t in range(NT):
                    sft = sfp.tile([P, N2], BF16, tag=f"sf{t}",
                                   name=f"{fx}sf_{t}")
                    nc.vector.tensor_single_scalar(
                        sft[:], iota1024[:], F_all[:, t:t + 1], AL.is_equal)
                    nc.sync.dma_start(sv2[0, t], sft[:])
                    sf_tiles.append(sft)
                for qt in range(8):
                    sft = sfp.tile([P, N2], BF16, tag=f"sf{NT + qt}",
                                   name=f"{fx}sf_m1_{qt}")
                    nc.vector.tensor_single_scalar(
                        sft[:], iota1024[:], idx2f[:, qt:qt + 1],
                        AL.is_equal)
                    nc.sync.dma_start(sv4[1, qt], sft[:])
                    sf_tiles.append(sft)
                # identity rows (token 512q+4p+3 -> dst 128q+p)
                for qt in range(8):
                    sid = sidp.tile([P, N2], BF16, tag="sid",
                                    name=f"{fx}sid{qt}")
                    nc.vector.memset(sid[:], 0.0)
                    nc.scalar.copy(sid[:, qt * P:(qt + 1) * P], ident_bf[:])
                    nc.sync.dma_start(sv4[3, qt], sid[:])
                zrow = tb.tile([1, N2], BF16)
                nc.vector.memset(zrow[:], 0.0)
                nc.sync.dma_start(s_dram[PTOK:PTOK + 1], zrow[:])
                # counts
                cnt_row = tb.tile([1, N2], F32)
                for jc in range(2):
                    pc = psT.tile([1, 512], F32, tag="cnt",
                                  name=f"{fx}cnt{jc}")
                    for t in range(NT + 8):
                        nc.tensor.matmul(
                            pc[:], ones_col_bf[:],
                            sf_tiles[t][:, jc * 512:(jc + 1) * 512],
                            start=(t == 0), stop=(t == NT + 7),
                            skip_group_check=True)
                    nc.scalar.copy(cnt_row[:, jc * 512:(jc + 1) * 512],
                                   pc[:])
                nc.vector.tensor_scalar_add(cnt_row[:], cnt_row[:], 1.0)
                nc.sync.dma_start(cnt_dram[:][None, :], cnt_row[:])
                cnt_col = tb.tile([P, 8], F32)
                nc.sync.dma_start(
                    cnt_col[:],
                    cnt_dram[:].rearrange("(b p) -> p b", b=8, p=P))
                nc.vector.reciprocal(s2r[:], cnt_col[:])

            # ---- bucket + rank + slot ----
            blk_all = tb.tile([P, 2 * NT], F32)
            tmp_b = tb.tile([P, 2 * NT], F32)
            nc.vector.tensor_scalar(blk_all[:], F_all[:], float(P), 0.0,
                                    AL.is_ge, AL.add)
            for b in range(2, 8):
                nc.vector.tensor_scalar(tmp_b[:], F_all[:], float(P * b),
                                        0.0, AL.is_ge, AL.add)
                nc.vector.tensor_add(blk_all[:], blk_all[:], tmp_b[:])
            bv = blk_dram[:].rearrange("(c p o) -> o p c", c=NT, p=P, o=2)
            nc.sync.dma_start(bv[0], blk_all[:, 0:NT])
            nc.sync.dma_start(bv[1], blk_all[:, NT:2 * NT])
            blk_row = tb.tile([1, PTOK], F32)
            nc.sync.dma_start(blk_row[:], blk_dram[:][None, :])
            # broadcast to 8 partitions (chunked), one-hot match, prefix scan
            m8 = tb.tile([8, PTOK], F32)
            r8 = tb.tile([8, PTOK], F32)
            z1 = tb.tile([8, 512], F32)
            nc.vector.memset(z1[:], 0.0)
            for jc in range(8):
                pb = psT.tile([8, 512], F32, tag="bc8", name=f"{fx}b8_{jc}")
                nc.tensor.matmul(pb[:], ones_row1[0:1, 0:8],
                                 blk_row[:, jc * 512:(jc + 1) * 512],
                                 start=True, stop=True)
                nc.vector.tensor_single_scalar(
                    m8[:, jc * 512:(jc + 1) * 512], pb[:], iota8[:, 0:1],
                    AL.is_equal)
            for jc in range(8):
                nc.vector.tensor_tensor_scan(
                    r8[:, jc * 512:(jc + 1) * 512],
                    m8[:, jc * 512:(jc + 1) * 512], z1[:],
                    0.0 if jc == 0 else r8[:, jc * 512 - 1:jc * 512],
                    AL.add, AL.add)
            nc.vector.tensor_mul(m8[:], m8[:], r8[:])
            slot_row = tb.tile([1, PTOK], F32)
            nc.vector.tensor_scalar(slot_row[:], blk_row[:], 640.0, -1.0,
                                    AL.mult, AL.add)
            for jc in range(8):
                pr = psT.tile([1, 512], F32, tag="rk", name=f"{fx}rk_{jc}")
                nc.tensor.matmul(pr[:], ones_col[0:8, 0:1],
                                 m8[:, jc * 512:(jc + 1) * 512],
                                 start=True, stop=True)
                nc.vector.tensor_add(slot_row[:, jc * 512:(jc + 1) * 512],
                                     slot_row[:, jc * 512:(jc + 1) * 512],
                                     pr[:])

            # ---- scatter payload (token id + 1), zero table, scatter ----
            nc.sync.dma_start(slot_dram[:][None, :], slot_row[:])
            sl16f = tb.tile([P, PTOK // 16], F32)
            nc.sync.dma_start(
                sl16f[0:16, :],
                slot_dram[:].rearrange("(s p) -> p s", p=16))
            sl16 = tb.tile([P, PTOK // 16], I16)
            nc.vector.tensor_copy(sl16[0:16, :], sl16f[0:16, :])
            for lo, n in ((16, 16), (32, 32), (64, 64)):
                nc.sync.dma_start(sl16[lo:lo + n, :], sl16[0:n, :])
            pay = tb.tile([P, 32 * 64], F32)
            nc.vector.memset(pay[:], 0.0)
            nc.gpsimd.iota(pay[:, 0::64], pattern=[[P, 32]], base=1,
                           channel_multiplier=1,
                           allow_small_or_imprecise_dtypes=True)
            zt = tb.tile([P, 640], F32)
            nc.vector.memset(zt[:], 0.0)
            gz = gidx_dram[:].rearrange("(p q) e -> p (q e)", p=P, q=40)
            for zc in range(4):
                nc.sync.dma_start(gz[:, zc * 640:(zc + 1) * 640], zt[:])
            nc.gpsimd.dma_scatter_add(
                gidx_dram[:],
                pay[:].rearrange("p (q e) -> p q e", q=32, e=64),
                sl16[:], PTOK, PTOK, 64)
            # ---- read back in 4 chunks, build int16 gather tables ----
            urow = tb.tile([16, 320], F32)
            with tc.tile_pool(name=f"{fx}gch", bufs=2) as gchp:
                for ch in range(8):
                    gch = gchp.tile([16, 40 * 64], F32, tag="gch",
                                    name=f"{fx}gch{ch}")
                    gv = gidx_dram[:].rearrange("(g c p) e -> g p c e",
                                                g=8, c=40, p=16)
                    nc.sync.dma_start(
                        gch[:].rearrange("p (c e) -> p c e", c=40, e=64),
                        gv[ch])
                    nc.vector.tensor_scalar_add(
                        urow[:, ch * 40:(ch + 1) * 40], gch[:, 0::64], -1.0)
            xif = tb.tile([16, 320], F32)
            nc.vector.tensor_scalar_max(xif[:], urow[:], 0.0)
            sif = tb.tile([16, 320], F32)
            nc.vector.tensor_scalar(sif[:], urow[:], 0.0, float(PTOK + 1),
                                    AL.is_lt, AL.mult)
            nc.vector.tensor_add(sif[:], sif[:], urow[:])
            nc.vector.tensor_copy(idxT_x[0:16, :], xif[:])
            nc.vector.tensor_copy(idxT_s[0:16, :], sif[:])
            for lo, n in ((16, 16), (32, 32), (64, 64)):
                nc.sync.dma_start(idxT_x[lo:lo + n, :], idxT_x[0:n, :])
                nc.sync.dma_start(idxT_s[lo:lo + n, :], idxT_s[0:n, :])
            if lvl == 4:
                nc.sync.dma_start(out[0:1, 0:PTOK], slot_row[:])
                nc.sync.dma_start(out[1:2, 0:PTOK], blk_row[:])
                nc.sync.dma_start(out[2:18, 0:320], xif[:])
                nc.sync.dma_start(out[18:34, 0:320], sif[:])
                nc.sync.dma_start(out[40:168, 0:32], pay[:, 0::64])
                nc.sync.dma_start(out[170:186, 0:256], sl16f[0:16, :])
                nc.sync.dma_start(out[190:206, 0:320], urow[:])
        if lvl == 4:
            return

        # ---- per-bucket gather + matmul + scale + out ----
        with (
            tc.tile_pool(name=f"{fx}xg", bufs=2) as xgp,
            tc.tile_pool(name=f"{fx}sg", bufs=2) as sgp,
            tc.tile_pool(name=f"{fx}oe", bufs=3) as oe,
            tc.tile_pool(name=f"{fx}psE", bufs=2, space="PSUM") as psE,
        ):
            for b in range(8):
                xg = xgp.tile([P, 5 * C], BF16, tag="xg", name=f"{fx}xg{b}")
                nc.gpsimd.dma_gather(
                    xg[:].rearrange("p (k c) -> p k c", k=5, c=C),
                    echo[:], idxT_x[:, 40 * b:40 * (b + 1)],
                    640, 640, C, elem_step=C)
                sg = sgp.tile([P, 5 * P], BF16, tag="sg", name=f"{fx}sg{b}")
                nc.gpsimd.dma_gather(
                    sg[:].rearrange("p (k c) -> p k c", k=5, c=P),
                    s_dram[:, b * P:(b + 1) * P],
                    idxT_s[:, 40 * b:40 * (b + 1)],
                    640, 640, P, elem_step=N2)
                for h2 in range(2):
                    acc = psE.tile([P, N1], F32, tag="acc",
                                   name=f"{fx}acc_{b}_{h2}")
                    for k in range(5):
                        for q in range(4):
                            nc.tensor.matmul(
                                acc[:, q * 512:(q + 1) * 512],
                                sg[:, k * P:(k + 1) * P],
                                xg[:, k * C + h2 * N1 + q * 512:
                                   k * C + h2 * N1 + (q + 1) * 512],
                                start=(k == 0), stop=(k == 4),
                                skip_group_check=True)
                    osb = oe.tile([P, N1], F32, tag="osb",
                                  name=f"{fx}osb_{b}_{h2}")
                    nc.vector.tensor_scalar_mul(osb[:], acc[:],
                                                s2r[:, b:b + 1])
                    nc.sync.dma_start(
                        out[b * P:(b + 1) * P, h2 * N1:(h2 + 1) * N1],
                        osb[:])


_CACHED = None


def kernel(x: np.ndarray, target_num_token=None) -> np.ndarray:
    global _CACHED
    x = np.ascontiguousarray(np.asarray(x), dtype=np.float32)
    b = x.shape[0]
    assert x.shape == (8, PTOK, C), x.shape
    if _CACHED is None:
        _CACHED = build_kernel()
    nc = _CACHED
    in_maps = [{"x": x[i]} for i in range(b)]
    res = run_bass_kernel_spmd(nc, in_maps, core_ids=list(range(b)))
    return np.stack([res.results[i]["out"] for i in range(b)])


# revision 4
# speedup vs baseline: 1.0538x; 1.0538x over previous
"""v3: v2 + bf16 x-echo in phase A + gather-based scatter phase E.

E': tokens bucketed by dst block (F>>7). Rank-within-bucket via DVE prefix
scan; slot = 640*bucket + rank - 1; inverse table via dma_scatter_add of
256B payload rows (token_id+1) into a zeroed [5120, 64] DRAM table; read
back, clamp dummies (x-gather -> row 0 masked by zero S; S-gather -> zeros
row 4096). Per bucket: dma_gather 640 x rows (bf16 echo) + 640 S-slices
(one-hot rows restricted to the bucket's 128 dst cols); 5 stationary
matmuls per 512-chunk accumulate PSUM; scale by 1/count; DMA out.
PE cost ~205k cycles vs ~1M for the dense one-hot scatter.
"""

import numpy as np

import concourse.bacc as bacc
import concourse.bass as bass
import concourse.mybir as mybir
import concourse.tile as tile
from concourse.bass_utils import run_bass_kernel_spmd
from concourse.masks import make_identity

F32 = mybir.dt.float32
BF16 = mybir.dt.bfloat16
F16 = mybir.dt.float16
U32 = mybir.dt.uint32
I16 = mybir.dt.int16

P = 128
NT = 16
C = 4096
D = 128
PTOK = 4096
N1 = 2048
N2 = 1024
NSLOT = 5120      # 8 buckets x 640 slots
AL = mybir.AluOpType

_LVL = {"A": 0, "B": 1, "C": 2, "D": 3, "T": 4, "E": 5}


def _newton_rsqrt(nc, pool, y, n2, shape, pfx):
    t = pool.tile(shape, F32, tag=f"{pfx}newt", name=f"{pfx}newt")
    for _ in range(2):
        nc.vector.tensor_mul(t[:], y, y)
        nc.vector.tensor_mul(t[:], t[:], n2)
        nc.vector.tensor_scalar(t[:], t[:], -0.5, 1.5, AL.mult, AL.add)
        nc.vector.tensor_mul(y, y, t[:])


def build_kernel(stop_after="E", reps=1):
    lvl = _LVL[stop_after]
    nc = bacc.Bacc(None, target_bir_lowering=False)
    x = nc.dram_tensor("x", [PTOK, C], F32, kind="ExternalInput")
    out = nc.dram_tensor("out", [N2, C], F32, kind="ExternalOutput")
    g_dram = nc.dram_tensor("g_scratch", [N1], F32, kind="Internal")
    cnt_dram = nc.dram_tensor("cnt_scratch", [N2], F32, kind="Internal")
    i1_dram = nc.dram_tensor("i1_scratch", [N1], F32, kind="Internal")
    blk_dram = nc.dram_tensor("blk_scratch", [PTOK], F32, kind="Internal")
    slot_dram = nc.dram_tensor("slot_scratch", [PTOK], F32, kind="Internal")
    f_dram = nc.dram_tensor("f_scratch", [PTOK], F32, kind="Internal")
    gidx_dram = nc.dram_tensor("gidx_scratch", [NSLOT, 64], F32,
                               kind="Internal")
    echo = nc.dram_tensor("echo", [PTOK, C], BF16, kind="Internal")
    s_dram = nc.dram_tensor("s_onehot", [PTOK + 1, N2], BF16, kind="Internal")

    xv = x[:].rearrange("(t p o) c -> o t p c", t=NT, p=P, o=2)
    ev = echo[:].rearrange("(t p o) c -> o t p c", t=NT, p=P, o=2)
    sv2 = s_dram[0:PTOK].rearrange("(t p o) c -> o t p c", t=NT, p=P, o=2)
    sv4 = s_dram[0:PTOK].rearrange("(q p r) c -> r q p c", q=8, p=P, r=4)

    with tile.TileContext(nc) as tc:
        with tc.tile_pool(name="const", bufs=1) as cpool:
            cst = {}
            ident = cpool.tile([P, P], F32)
            make_identity(nc, ident[:])
            cst["ident"] = ident
            ident_bf = cpool.tile([P, P], BF16)
            nc.vector.tensor_copy(ident_bf[:], ident[:])
            cst["ident_bf"] = ident_bf
            iota2048 = cpool.tile([P, N1], F32)
            nc.gpsimd.iota(iota2048[:], pattern=[[1, N1]], base=0,
                           channel_multiplier=0,
                           allow_small_or_imprecise_dtypes=True)
            cst["iota2048"] = iota2048
            iota1024 = cpool.tile([P, N2], F32)
            nc.gpsimd.iota(iota1024[:], pattern=[[1, N2]], base=0,
                           channel_multiplier=0,
                           allow_small_or_imprecise_dtypes=True)
            cst["iota1024"] = iota1024
            ones_col = cpool.tile([P, 1], F32)
            nc.vector.memset(ones_col[:], 1.0)
            cst["ones_col"] = ones_col
            ones_col_bf = cpool.tile([P, 1], BF16)
            nc.vector.memset(ones_col_bf[:], 1.0)
            cst["ones_col_bf"] = ones_col_bf
            ones_row1 = cpool.tile([1, P], F32)
            nc.vector.memset(ones_row1[:], 1.0)
            cst["ones_row1"] = ones_row1
            iota_pcol = cpool.tile([P, NT], F32)
            nc.gpsimd.iota(iota_pcol[:], pattern=[[P, NT]], base=0,
                           channel_multiplier=1,
                           allow_small_or_imprecise_dtypes=True)
            cst["iota_pcol"] = iota_pcol
            iota8 = cpool.tile([8, 1], F32)
            nc.gpsimd.iota(iota8[:], pattern=[[0, 1]], base=0,
                           channel_multiplier=1,
                           allow_small_or_imprecise_dtypes=True)
            cst["iota8"] = iota8
            cst["f_dram"] = f_dram
            for rep in range(reps):
                _body(nc, tc, lvl, f"r{rep}_" if reps > 1 else "",
                      out, g_dram, cnt_dram, i1_dram, blk_dram, slot_dram,
                      gidx_dram, echo, s_dram, xv, ev, sv2, sv4, cst)
    nc.finalize()
    return nc


def _body(nc, tc, lvl, fx, out, g_dram, cnt_dram, i1_dram, blk_dram,
          slot_dram, gidx_dram, echo, s_dram, xv, ev, sv2, sv4, cst):
    with tc.tile_pool(name=f"{fx}small", bufs=1) as spool:
        idx1f = spool.tile([P, NT], F32)
        F_all = spool.tile([P, 2 * NT], F32)
        s2r = spool.tile([P, 8], F32)
        idx2f = spool.tile([P, 8], F32)
        done = _phases_abcd(nc, tc, lvl, fx, out, g_dram, i1_dram, xv, ev,
                            cst, spool, idx1f, F_all, idx2f)
        if not done:
            return
        _phase_e(nc, tc, lvl, fx, out, cnt_dram, blk_dram, slot_dram,
                 gidx_dram, echo, s_dram, sv2, sv4, cst, F_all, idx2f, s2r)


def _phases_abcd(nc, tc, lvl, fx, out, g_dram, i1_dram, xv, ev, cst,
                 spool, idx1f, F_all, idx2f):
    ident = cst["ident"]
    iota2048 = cst["iota2048"]
    ones_col = cst["ones_col"]
    ones_row1 = cst["ones_row1"]
    iota_pcol = cst["iota_pcol"]

    def head_sum(pool, xt, nm):
        """Out-of-place first level (keeps xt intact for the echo DMA),
        then in-place tree on the half-size scratch."""
        ht = pool.tile([P, N1], F32, tag="ht", name=nm)
        nc.vector.tensor_add(ht[:], xt[:, :N1], xt[:, N1:])
        w = N1
        while w > D:
            h = w // 2
            nc.vector.tensor_add(ht[:, :h], ht[:, :h], ht[:, h:w])
            w = h
        return ht

    with tc.tile_pool(name=f"{fx}big", bufs=1) as bpool:
        mx_even = bpool.tile([P, N1], F32)
        mxT_even = bpool.tile([P, N1], F32)
        mxT_odd = bpool.tile([P, N1], F32)
        mxT_odd_n = bpool.tile([P, N1], F32)
        macc_eT = bpool.tile([P, N2], F32)
        macc_oT = bpool.tile([P, N2], F32)
        macc_oT_n = bpool.tile([P, N2], F32)
        idx_b1 = spool.tile([P, NT * 8], U32)
        idx_b2 = spool.tile([P, 8 * 8], U32)

        # ========== A-odd ==========
        with (
            tc.tile_pool(name=f"{fx}xa", bufs=4) as xa,
            tc.tile_pool(name=f"{fx}ho", bufs=4) as ho,
            tc.tile_pool(name=f"{fx}psA", bufs=2, space="PSUM") as psA,
        ):
            for ti in range(NT):
                xt = xa.tile([P, C], F32, tag="xt", name=f"{fx}xo{ti}")
                nc.sync.dma_start(xt[:], xv[1, ti])
                nc.gpsimd.dma_start(ev[1, ti], xt[:])
                ht = head_sum(ho, xt, f"{fx}ho{ti}")
                pt = psA.tile([P, P], F32, tag="tr", name=f"{fx}tro{ti}")
                nc.tensor.transpose(pt[:], ht[:, :D], ident[:])
                nc.scalar.copy(mxT_odd[:, ti * D:(ti + 1) * D], pt[:])

            with (
                tc.tile_pool(name=f"{fx}nb", bufs=1) as nb,
                tc.tile_pool(name=f"{fx}psN", bufs=2, space="PSUM") as psN,
            ):
                sq = nb.tile([P, N1], F32)
                nc.scalar.activation(sq[:], mxT_odd[:],
                                     mybir.ActivationFunctionType.Square)
                n2row = nb.tile([1, N1], F32)
                for jc in range(4):
                    pn = psN.tile([1, 512], F32, tag="n2",
                                  name=f"{fx}n2_{jc}")
                    nc.tensor.matmul(pn[:], ones_col[:],
                                     sq[:, jc * 512:(jc + 1) * 512],
                                     start=True, stop=True)
                    nc.scalar.copy(n2row[:, jc * 512:(jc + 1) * 512], pn[:])
                rinv = nb.tile([1, N1], F32)
                sqr = nb.tile([1, N1], F32)
                nc.scalar.activation(sqr[:], n2row[:],
                                     mybir.ActivationFunctionType.Sqrt)
                nc.vector.reciprocal(rinv[:], sqr[:])
                _newton_rsqrt(nc, nb, rinv[:], n2row[:], [1, N1], fx)
                for jc in range(4):
                    pb = psN.tile([P, 512], F32, tag="bc",
                                  name=f"{fx}bc_{jc}")
                    nc.tensor.matmul(pb[:], ones_row1[:],
                                     rinv[:, jc * 512:(jc + 1) * 512],
                                     start=True, stop=True)
                    nc.vector.tensor_mul(
                        mxT_odd_n[:, jc * 512:(jc + 1) * 512],
                        mxT_odd[:, jc * 512:(jc + 1) * 512], pb[:])

            # ===== A-even + scores1 + pmacc =====
            with (
                tc.tile_pool(name=f"{fx}sc1", bufs=2) as sc1,
                tc.tile_pool(name=f"{fx}ps1", bufs=2, space="PSUM") as ps1,
                tc.tile_pool(name=f"{fx}s1p", bufs=2) as s1p,
                tc.tile_pool(name=f"{fx}psM", bufs=1, space="PSUM") as psM,
            ):
                m8 = spool.tile([P, 8], F32)
                pmacc = psM.tile([P, N1], F32, tag="macc")
                for ti in range(NT):
                    xt = xa.tile([P, C], F32, tag="xt", name=f"{fx}xe{ti}")
                    nc.sync.dma_start(xt[:], xv[0, ti])
                    nc.gpsimd.dma_start(ev[0, ti], xt[:])
                    ht = head_sum(ho, xt, f"{fx}he{ti}")
                    nc.scalar.copy(mx_even[:, ti * D:(ti + 1) * D],
                                   ht[:, :D])
                    pt = psA.tile([P, P], F32, tag="tr", name=f"{fx}tre{ti}")
                    nc.tensor.transpose(pt[:], ht[:, :D], ident[:])
                    nc.scalar.copy(mxT_even[:, ti * D:(ti + 1) * D], pt[:])
                    ssb = sc1.tile([P, N1], F32, tag="ssb",
                                   name=f"{fx}sb{ti}")
                    for jc in range(4):
                        psc = ps1.tile([P, 512], F32, tag="sc",
                                       name=f"{fx}sc{ti}_{jc}")
                        nc.tensor.matmul(
                            psc[:], mxT_even[:, ti * D:(ti + 1) * D],
                            mxT_odd_n[:, jc * 512:(jc + 1) * 512],
                            start=True, stop=True)
                        nc.scalar.copy(ssb[:, jc * 512:(jc + 1) * 512],
                                       psc[:])
                    nc.vector.max(m8[:], ssb[:])
                    nc.vector.max_index(idx_b1[:, ti * 8:(ti + 1) * 8],
                                        m8[:], ssb[:])
                    nc.vector.tensor_copy(idx1f[:, ti:ti + 1],
                                          idx_b1[:, ti * 8:ti * 8 + 1])
                    s1t = s1p.tile([P, N1], F32, tag="s1",
                                   name=f"{fx}s1_{ti}")
                    nc.vector.tensor_single_scalar(
                        s1t[:], iota2048[:], idx1f[:, ti:ti + 1],
                        AL.is_equal)
                    for jc in range(4):
                        nc.tensor.matmul(
                            pmacc[:, jc * 512:(jc + 1) * 512],
                            mx_even[:, ti * D:(ti + 1) * D],
                            s1t[:, jc * 512:(jc + 1) * 512],
                            start=(ti == 0), stop=(ti == NT - 1),
                            skip_group_check=True)
                if lvl == 0:
                    nc.sync.dma_start(out[0:P, 0:N1], mx_even[:])
                    nc.sync.dma_start(out[0:P, N1:2 * N1], mxT_even[:])
                    nc.sync.dma_start(out[P:2 * P, 0:N1], mxT_odd[:])
                if lvl == 1:
                    nc.sync.dma_start(out[0:P, 0:NT], idx1f[:])
                    nc.sync.dma_start(out[0:P, 32:32 + N1], mxT_odd_n[:])
                if lvl < 2:
                    return False
                nc.vector.tensor_add(macc_eT[:], pmacc[:, ::2],
                                     mxT_odd[:, ::2])
                nc.vector.tensor_add(macc_oT[:], pmacc[:, 1::2],
                                     mxT_odd[:, 1::2])

            with (
                tc.tile_pool(name=f"{fx}nb2", bufs=1) as nb2,
                tc.tile_pool(name=f"{fx}psN2", bufs=2, space="PSUM") as psN2,
            ):
                sq2 = nb2.tile([P, N2], F32)
                nc.scalar.activation(sq2[:], macc_oT[:],
                                     mybir.ActivationFunctionType.Square)
                n2row2 = nb2.tile([1, N2], F32)
                for jc in range(2):
                    pn = psN2.tile([1, 512], F32, tag="n2",
                                   name=f"{fx}n2b_{jc}")
                    nc.tensor.matmul(pn[:], ones_col[:],
                                     sq2[:, jc * 512:(jc + 1) * 512],
                                     start=True, stop=True)
                    nc.scalar.copy(n2row2[:, jc * 512:(jc + 1) * 512],
                                   pn[:])
                rinv2 = nb2.tile([1, N2], F32)
                sqr2 = nb2.tile([1, N2], F32)
                nc.scalar.activation(sqr2[:], n2row2[:],
                                     mybir.ActivationFunctionType.Sqrt)
                nc.vector.reciprocal(rinv2[:], sqr2[:])
                _newton_rsqrt(nc, nb2, rinv2[:], n2row2[:], [1, N2],
                              fx + "b")
                for jc in range(2):
                    pb = psN2.tile([P, 512], F32, tag="bc",
                                   name=f"{fx}bcb_{jc}")
                    nc.tensor.matmul(pb[:], ones_row1[:],
                                     rinv2[:, jc * 512:(jc + 1) * 512],
                                     start=True, stop=True)
                    nc.vector.tensor_mul(
                        macc_oT_n[:, jc * 512:(jc + 1) * 512],
                        macc_oT[:, jc * 512:(jc + 1) * 512], pb[:])

            with (
                tc.tile_pool(name=f"{fx}sc2", bufs=2) as sc2,
                tc.tile_pool(name=f"{fx}ps2", bufs=2, space="PSUM") as ps2,
            ):
                m8b = spool.tile([P, 8], F32)
                for t2 in range(8):
                    ssb2 = sc2.tile([P, N2], F32, tag="ssb2",
                                    name=f"{fx}sb2_{t2}")
                    for jc in range(2):
                        psc = ps2.tile([P, 512], F32, tag="sc2",
                                       name=f"{fx}sc2_{t2}_{jc}")
                        nc.tensor.matmul(
                            psc[:], macc_eT[:, t2 * D:(t2 + 1) * D],
                            macc_oT_n[:, jc * 512:(jc + 1) * 512],
                            start=True, stop=True)
                        nc.scalar.copy(ssb2[:, jc * 512:(jc + 1) * 512],
                                       psc[:])
                    nc.vector.max(m8b[:], ssb2[:])
                    nc.vector.max_index(idx_b2[:, t2 * 8:(t2 + 1) * 8],
                                        m8b[:], ssb2[:])
            nc.vector.tensor_copy(idx2f[:], idx_b2[:, ::8])
            if lvl == 2:
                nc.sync.dma_start(out[0:P, 0:8], idx2f[:])
                nc.sync.dma_start(out[0:P, 8:8 + N2], macc_eT[:])
                nc.sync.dma_start(out[P:2 * P, 0:N2], macc_oT[:])
            if lvl < 3:
                return False

            # ================= D: compose F =================
            with (
                tc.tile_pool(name=f"{fx}cmp", bufs=1) as cmp,
                tc.tile_pool(name=f"{fx}s1d", bufs=2) as s1d,
                tc.tile_pool(name=f"{fx}psD", bufs=1, space="PSUM") as psD,
            ):
                g_row = cmp.tile([1, N1], F32)
                nc.gpsimd.iota(g_row[0:1, 1::2], pattern=[[1, N2]], base=0,
                               channel_multiplier=0,
                               allow_small_or_imprecise_dtypes=True)
                gv = g_dram[:].rearrange("(t p o) -> o p t", t=8, p=P, o=2)
                nc.sync.dma_start(gv[0], idx2f[:])
                gk = g_dram[:].rearrange("(k o) -> o k", o=2)
                nc.sync.dma_start(g_row[0:1, 0::2], gk[0][None, :])
                nc.sync.dma_start(g_dram[:][None, :], g_row[:])
                gf = g_dram[:].rearrange("(t p) -> p t", t=NT, p=P)
                nc.sync.dma_start(F_all[:, NT:2 * NT], gf)
                i1d = i1_dram[:].rearrange("(t p) -> p t", t=NT, p=P)
                nc.sync.dma_start(i1d, idx1f[:])
                i1row = cmp.tile([1, N1], F32)
                nc.sync.dma_start(i1row[:], i1_dram[:][None, :])
                idx1_bc = cmp.tile([P, N1], F32)
                for jc in range(4):
                    pb = psD.tile([P, 512], F32, tag="gb",
                                  name=f"{fx}gb{jc}")
                    nc.tensor.matmul(pb[:], ones_row1[:],
                                     i1row[:, jc * 512:(jc + 1) * 512],
                                     start=True, stop=True)
                    nc.scalar.copy(idx1_bc[:, jc * 512:(jc + 1) * 512],
                                   pb[:])
                g16 = cmp.tile([P, NT], F16)
                nc.vector.tensor_copy(g16[:], F_all[:, NT:2 * NT])
                pfr = [psD.tile([1, 512], F32, tag=f"pfr{c}",
                                name=f"{fx}pfr{c}") for c in range(4)]
                for jt in range(NT):
                    s1tt = s1d.tile([P, N1], F16, tag="s1d",
                                    name=f"{fx}s1tt_{jt}")
                    nc.vector.tensor_single_scalar(
                        s1tt[:], idx1_bc[:], iota_pcol[:, jt:jt + 1],
                        AL.is_equal)
                    for ic in range(4):
                        nc.tensor.matmul(
                            pfr[ic][:], g16[:, jt:jt + 1],
                            s1tt[:, ic * 512:(ic + 1) * 512],
                            start=(jt == 0), stop=(jt == NT - 1),
                            skip_group_check=True)
                fe_row = cmp.tile([1, N1], F32)
                for ic in range(4):
                    nc.scalar.copy(fe_row[:, ic * 512:(ic + 1) * 512],
                                   pfr[ic][:])
                nc.sync.dma_start(i1_dram[:][None, :], fe_row[:])
                nc.sync.dma_start(
                    F_all[:, 0:NT],
                    i1_dram[:].rearrange("(t p) -> p t", t=NT, p=P))
            if lvl == 3:
                nc.sync.dma_start(out[0:P, 0:2 * NT], F_all[:])
            return lvl >= 4


def _phase_e(nc, tc, lvl, fx, out, cnt_dram, blk_dram, slot_dram, gidx_dram,
             echo, s_dram, sv2, sv4, cst, F_all, idx2f, s2r):
    iota1024 = cst["iota1024"]
    ones_col = cst["ones_col"]
    ones_col_bf = cst["ones_col_bf"]
    ones_row1 = cst["ones_row1"]
    iota8 = cst["iota8"]
    f_dram = cst["f_dram"]

    with tc.tile_pool(name=f"{fx}ix", bufs=1) as ixp:
        # idx table (HW wrap layout: 16-partition wrap replicated x8), and
        # the on-chip one-hot stationaries built from the F readback.
        idxT_x = ixp.tile([P, 320], I16)
        sg_tiles = [ixp.tile([P, 5 * P], BF16, tag=f"sg{b}",
                             name=f"{fx}sg{b}") for b in range(8)]
        # off the critical path: payload iota + gidx table zero-fill
        pay = ixp.tile([P, 32 * 64], F32)
        nc.vector.memset(pay[:], 0.0)
        nc.gpsimd.iota(pay[:, 0::64], pattern=[[P, 32]], base=1,
                       channel_multiplier=1,
                       allow_small_or_imprecise_dtypes=True)
        zt0 = ixp.tile([P, 640], F32)
        nc.vector.memset(zt0[:], 0.0)
        gz = gidx_dram[:].rearrange("(p q) e -> p (q e)", p=P, q=40)
        for zc in range(4):
            nc.sync.dma_start(gz[:, zc * 640:(zc + 1) * 640], zt0[:])

        with (
            tc.tile_pool(name=f"{fx}tb", bufs=1) as tb,
            tc.tile_pool(name=f"{fx}psT", bufs=2, space="PSUM") as psT,
        ):
            # ---- bucket + rank + slot ----
            blk_all = tb.tile([P, 2 * NT], F32)
            tmp_b = tb.tile([P, 2 * NT], F32)
            nc.vector.tensor_scalar(blk_all[:], F_all[:], float(P), 0.0,
                                    AL.is_ge, AL.add)
            for b in range(2, 8):
                nc.vector.tensor_scalar(tmp_b[:], F_all[:], float(P * b),
                                        0.0, AL.is_ge, AL.add)
                nc.vector.tensor_add(blk_all[:], blk_all[:], tmp_b[:])
            bv = blk_dram[:].rearrange("(c p o) -> o p c", c=NT, p=P, o=2)
            nc.sync.dma_start(bv[0], blk_all[:, 0:NT])
            nc.sync.dma_start(bv[1], blk_all[:, NT:2 * NT])
            blk_row = tb.tile([1, PTOK], F32)
            nc.sync.dma_start(blk_row[:], blk_dram[:][None, :])
            # F+1 into payload elem 1 (token-id payload order), via the same
            # DRAM bounce (blk_dram reused after blk_row is read)
            fv = f_dram[:].rearrange("(c p o) -> o p c", c=NT, p=P, o=2)
            nc.sync.dma_start(fv[0], F_all[:, 0:NT])
            nc.sync.dma_start(fv[1], F_all[:, NT:2 * NT])
            f_pay = tb.tile([P, 32], F32)
            nc.sync.dma_start(
                f_pay[:], f_dram[:].rearrange("(q p) -> p q", p=P, q=32))
            nc.vector.tensor_scalar_add(pay[:, 1::64], f_pay[:], 1.0)
            # broadcast to 8 partitions (chunked), one-hot match, prefix scan
            m8 = tb.tile([8, PTOK], F32)
            r8 = tb.tile([8, PTOK], F32)
            z1 = tb.tile([8, 512], F32)
            nc.vector.memset(z1[:], 0.0)
            for jc in range(8):
                pb = psT.tile([8, 512], F32, tag="bc8", name=f"{fx}b8_{jc}")
                nc.tensor.matmul(pb[:], ones_row1[0:1, 0:8],
                                 blk_row[:, jc * 512:(jc + 1) * 512],
                                 start=True, stop=True)
                nc.vector.tensor_single_scalar(
                    m8[:, jc * 512:(jc + 1) * 512], pb[:], iota8[:, 0:1],
                    AL.is_equal)
            for jc in range(8):
                nc.vector.tensor_tensor_scan(
                    r8[:, jc * 512:(jc + 1) * 512],
                    m8[:, jc * 512:(jc + 1) * 512], z1[:],
                    0.0 if jc == 0 else r8[:, jc * 512 - 1:jc * 512],
                    AL.add, AL.add)
            nc.vector.tensor_mul(m8[:], m8[:], r8[:])
            slot_row = tb.tile([1, PTOK], F32)
            nc.vector.tensor_scalar(slot_row[:], blk_row[:], 640.0, -1.0,
                                    AL.mult, AL.add)
            for jc in range(8):
                pr = psT.tile([1, 512], F32, tag="rk", name=f"{fx}rk_{jc}")
                nc.tensor.matmul(pr[:], ones_col[0:8, 0:1],
                                 m8[:, jc * 512:(jc + 1) * 512],
                                 start=True, stop=True)
                nc.vector.tensor_add(slot_row[:, jc * 512:(jc + 1) * 512],
                                     slot_row[:, jc * 512:(jc + 1) * 512],
                                     pr[:])

            # ---- wrapped int16 slot table, scatter ----
            nc.sync.dma_start(slot_dram[:][None, :], slot_row[:])
            sl16f = tb.tile([P, PTOK // 16], F32)
            nc.sync.dma_start(
                sl16f[0:16, :],
                slot_dram[:].rearrange("(s p) -> p s", p=16))
            sl16 = tb.tile([P, PTOK // 16], I16)
            nc.vector.tensor_copy(sl16[0:16, :], sl16f[0:16, :])
            for lo, n in ((16, 16), (32, 32), (64, 64)):
                nc.sync.dma_start(sl16[lo:lo + n, :], sl16[0:n, :])
            nc.gpsimd.dma_scatter_add(
                gidx_dram[:],
                pay[:].rearrange("p (q e) -> p q e", q=32, e=64),
                sl16[:], PTOK, PTOK, 64)
            # ---- read back: token ids (wrap layout) + F (slot128 layout) --
            urow = tb.tile([16, 320], F32)
            with tc.tile_pool(name=f"{fx}gch", bufs=2) as gchp:
                for ch in range(8):
                    gch = gchp.tile([16, 40 * 64], F32, tag="gch",
                                    name=f"{fx}gch{ch}")
                    gv = gidx_dram[:].rearrange("(g c p) e -> g p c e",
                                                g=8, c=40, p=16)
                    nc.sync.dma_start(
                        gch[:].rearrange("p (c e) -> p c e", c=40, e=64),
                        gv[ch])
                    nc.vector.tensor_scalar_add(
                        urow[:, ch * 40:(ch + 1) * 40], gch[:, 0::64], -1.0)
            xif = tb.tile([16, 320], F32)
            nc.vector.tensor_scalar_max(xif[:], urow[:], 0.0)
            nc.vector.tensor_copy(idxT_x[0:16, :], xif[:])
            for lo, n in ((16, 16), (32, 32), (64, 64)):
                nc.sync.dma_start(idxT_x[lo:lo + n, :], idxT_x[0:n, :])
            fg = tb.tile([P, 40], F32)
            nc.sync.dma_start(
                fg[:].rearrange("j m e -> j m e" if False else "j (m e) -> j m e", m=40, e=1),
                gidx_dram[:].rearrange("(m j) e -> j m e",
                                       m=40, j=P)[:, :, 1:2])
            # one-hot stationaries + counts (per bucket), s2r
            floc = tb.tile([P, 40], F32)
            for b in range(8):
                nc.vector.tensor_scalar_add(floc[:, 5 * b:5 * (b + 1)],
                                            fg[:, 5 * b:5 * (b + 1)],
                                            float(-1 - P * b))
                for k in range(5):
                    nc.vector.tensor_single_scalar(
                        sg_tiles[b][:, k * P:(k + 1) * P],
                        iota1024[:, 0:P], floc[:, 5 * b + k:5 * b + k + 1],
                        AL.is_equal)
                pcnt = psT.tile([P, 1], F32, tag="pcnt", name=f"{fx}pc{b}")
                for k in range(5):
                    nc.tensor.matmul(pcnt[:],
                                     sg_tiles[b][:, k * P:(k + 1) * P],
                                     ones_col_bf[:],
                                     start=(k == 0), stop=(k == 4),
                                     skip_group_check=True)
                nc.vector.reciprocal(s2r[:, b:b + 1], pcnt[:])
            if lvl == 4:
                nc.sync.dma_start(out[0:1, 0:PTOK], slot_row[:])
                nc.sync.dma_start(out[1:2, 0:PTOK], blk_row[:])
                nc.sync.dma_start(out[2:18, 0:320], xif[:])
                nc.sync.dma_start(out[40:168, 0:40], fg[:])
        if lvl == 4:
            return

        # ---- per-bucket gather + matmul + scale + out ----
        with (
            tc.tile_pool(name=f"{fx}xg", bufs=2) as xgp,
            tc.tile_pool(name=f"{fx}oe", bufs=3) as oe,
            tc.tile_pool(name=f"{fx}psE", bufs=2, space="PSUM") as psE,
        ):
            for b in range(8):
                xg = xgp.tile([P, 5 * C], BF16, tag="xg", name=f"{fx}xg{b}")
                nc.gpsimd.dma_gather(
                    xg[:].rearrange("p (k c) -> p k c", k=5, c=C),
                    echo[:], idxT_x[:, 40 * b:40 * (b + 1)],
                    640, 640, C, elem_step=C)
                for h2 in range(2):
                    acc = psE.tile([P, N1], F32, tag="acc",
                                   name=f"{fx}acc_{b}_{h2}")
                    for k in range(5):
                        for q in range(4):
                            nc.tensor.matmul(
                                acc[:, q * 512:(q + 1) * 512],
                                sg_tiles[b][:, k * P:(k + 1) * P],
                                xg[:, k * C + h2 * N1 + q * 512:
                                   k * C + h2 * N1 + (q + 1) * 512],
                                start=(k == 0), stop=(k == 4),
                                skip_group_check=True)
                    osb = oe.tile([P, N1], F32, tag="osb",
                                  name=f"{fx}osb_{b}_{h2}")
                    nc.vector.tensor_scalar_mul(osb[:], acc[:],
                                                s2r[:, b:b + 1])
                    nc.sync.dma_start(
                        out[b * P:(b + 1) * P, h2 * N1:(h2 + 1) * N1],
                        osb[:])


_CACHED = None


def kernel(x: np.ndarray, target_num_token=None) -> np.ndarray:
    global _CACHED
    x = np.ascontiguousarray(np.asarray(x), dtype=np.float32)
    b = x.shape[0]
    assert x.shape == (8, PTOK, C), x.shape
    if _CACHED is None:
        _CACHED = build_kernel()
    nc = _CACHED
    in_maps = [{"x": x[i]} for i in range(b)]
    res = run_bass_kernel_spmd(nc, in_maps, core_ids=list(range(b)))
    return np.stack([res.results[i]["out"] for i in range(b)])
